# revision 2
# baseline (speedup 1.0000x reference)
"""HGNN (2-layer hetero GraphSAGE + 8 heads) on 8 trn2 NeuronCores.

Sharding: dst-node interleaved (core = v % 8, local = v // 8). Each layer is
one SPMD NEFF launch; the host performs the inter-layer halo exchange by
concatenating per-core outputs into fresh gather tables (indices are
pre-translated into the concatenated layout).

Device-side per layer, per core:
  - For each 512-dst-column PSUM group, edges (sorted by dst) are cut into
    128-edge windows on a column grid that is uniform across cores
    (min-over-cores advance), so a single program serves all 8 cores.
  - Per window: one indirect DMA gathers the 128 source rows [128, 128];
    a selection matrix sel[e, j] = (rel_dst[e] == j) * invcnt[e] is built
    with two batched DVE ops; PE accumulates g.T @ sel into the PSUM group,
    yielding the transposed scatter-mean m^T [128 feat, 512 dst] directly.
  - Dense stage: nb^T = Wl_bb.T @ m_bb^T + Wl_sb.T @ m_sb^T + Wr.T @ x^T,
    then bias + leaky-relu fused on the scalar engine. Head (layer 2) is one
    more matmul with Wh^T producing y^T [8, dst].
"""
import numpy as np

import concourse.bass as bass
import concourse.bacc as bacc
import concourse.mybir as mybir
import concourse.tile as tile
from concourse.bass_utils import run_bass_kernel_spmd

P = 128
D = 128
NCORES = 8
GROUP = 512       # psum columns per accumulation group
S = 32            # max dst-column span per 128-edge window
NB, NS = 100000, 50000
NLB, NLS = NB // NCORES, NS // NCORES   # 12500, 6250


# ---------------------------------------------------------------- host prep
def _shard_edges(src, dst, n_dst):
    """Split edges by dst core; per core return (src, dst_local) dst-sorted."""
    core = dst % NCORES
    loc = dst // NCORES
    out = []
    for c in range(NCORES):
        m = core == c
        s, d = src[m], loc[m]
        o = np.argsort(d, kind="stable")
        out.append((s[o].astype(np.int64), d[o].astype(np.int64)))
    return out


def _pack_type(per_core, n_loc):
    """Uniform-across-cores window packing.

    Returns (idx [NCORES,128,Wtot] i32, rel [NCORES,128,Wtot] f32,
             invc [NCORES,128,Wtot] f32, groups: list of list[(col_off, span)]).
    """
    ngroups = (n_loc + GROUP - 1) // GROUP
    # per-core prefix counts over columns
    counts = []
    for s, d in per_core:
        counts.append(np.bincount(d, minlength=n_loc))
    cum = [np.concatenate([[0], np.cumsum(c)]) for c in counts]  # [n_loc+1]
    invc_dst = [1.0 / np.maximum(c, 1) for c in counts]

    groups = []
    idx_cols, rel_cols, invc_cols = (
        [[] for _ in range(NCORES)],
        [[] for _ in range(NCORES)],
        [[] for _ in range(NCORES)],
    )
    for g in range(ngroups):
        c0, c1 = g * GROUP, min((g + 1) * GROUP, n_loc)
        wins = []
        c = c0
        while c < c1:
            span = min(S, c1 - c)
            # shrink span until every core has <= 128 edges in [c, c+span)
            while span > 1:
                ok = True
                for cc in range(NCORES):
                    if cum[cc][c + span] - cum[cc][c] > P:
                        ok = False
                        break
                if ok:
                    break
                span -= 1
            wins.append((c - c0, span))
            for cc in range(NCORES):
                s_arr, d_arr = per_core[cc]
                a, b = cum[cc][c], cum[cc][c + span]
                n = b - a
                assert n <= P, "single column exceeded 128 edges"
                icol = np.zeros(P, np.int32)
                rcol = np.full(P, -1.0, np.float32)
                vcol = np.zeros(P, np.float32)
                icol[:n] = s_arr[a:b]
                rcol[:n] = (d_arr[a:b] - c).astype(np.float32)
                vcol[:n] = invc_dst[cc][d_arr[a:b]].astype(np.float32)
                idx_cols[cc].append(icol)
                rel_cols[cc].append(rcol)
                invc_cols[cc].append(vcol)
            c += span
        groups.append(wins)

    idx = np.stack([np.stack(cols, 1) for cols in idx_cols])     # [NC, P, Wtot]
    rel = np.stack([np.stack(cols, 1) for cols in rel_cols])
    invc = np.stack([np.stack(cols, 1) for cols in invc_cols])
    return idx.astype(np.int32), rel.astype(np.float32), invc.astype(np.float32), groups


# ------------------------------------------------------------- device build
def _build_launch(cfg):
    """Build one layer's SPMD program. cfg keys:
      tabs: {name: nrows} gather tables
      types: list of dicts(name, tab, Wtot, groups, n_loc)
      head: bool — add 8-head output (layer 2)
      out_s: bool — emit s-node output (layer 1)
    """
    nc = bacc.Bacc("TRN2", target_bir_lowering=False, debug=False,
                   num_devices=NCORES)
    f32, i32 = mybir.dt.float32, mybir.dt.int32

    d_tab = {k: nc.dram_tensor(k, [n, D], f32, kind="ExternalInput")
             for k, n in cfg["tabs"].items()}
    d_xbT = nc.dram_tensor("xbT", [P, NLB], f32, kind="ExternalInput")
    d_xsT = (nc.dram_tensor("xsT", [P, NLS], f32, kind="ExternalInput")
             if cfg["out_s"] else None)
    # packed weights: Wl_bb | Wl_sb | Wr_b | [Wl_bs | Wr_s] | WhT | iota | biases
    nw = 3 * D + (2 * D if cfg["out_s"] else 0) + (8 if cfg["head"] else 0) + S + 3
    d_w = nc.dram_tensor("wts", [P, nw], f32, kind="ExternalInput")
    d_et = {}
    for t in cfg["types"]:
        W = t["Wtot"]
        d_et[t["name"]] = (
            nc.dram_tensor(f'idx_{t["name"]}', [P, W], i32, kind="ExternalInput"),
            nc.dram_tensor(f'rel_{t["name"]}', [P, W], f32, kind="ExternalInput"),
            nc.dram_tensor(f'ivc_{t["name"]}', [P, W], f32, kind="ExternalInput"),
        )
    d_nbT = nc.dram_tensor("nbT", [P, NLB], f32, kind="ExternalOutput")
    d_nsT = (nc.dram_tensor("nsT", [P, NLS], f32, kind="ExternalOutput")
             if cfg["out_s"] else None)
    d_yT = (nc.dram_tensor("yT", [8, NLB], f32, kind="ExternalOutput")
            if cfg["head"] else None)

    types = {t["name"]: t for t in cfg["types"]}

    from contextlib import ExitStack
    with tile.TileContext(nc) as tc, ExitStack() as ctx:
        wpool = ctx.enter_context(tc.tile_pool(name="w", bufs=1))
        gpool = ctx.enter_context(tc.tile_pool(name="g", bufs=12))
        mpool = ctx.enter_context(tc.tile_pool(name="m", bufs=3))
        spool = ctx.enter_context(tc.tile_pool(name="s", bufs=3))
        appool = ctx.enter_context(tc.tile_pool(name="ap", bufs=3, space="PSUM"))
        s2pool = ctx.enter_context(tc.tile_pool(name="s2", bufs=2, space="PSUM"))
        hpool = (ctx.enter_context(tc.tile_pool(name="h", bufs=2, space="PSUM"))
                 if cfg["head"] else None)

        t_w = wpool.tile([P, nw], f32)
        nc.sync.dma_start(t_w[:], d_w[:])
        off = 0
        w_Wlbb = t_w[:, off:off + D]; off += D
        w_Wlsb = t_w[:, off:off + D]; off += D
        w_Wrb = t_w[:, off:off + D]; off += D
        if cfg["out_s"]:
            w_Wlbs = t_w[:, off:off + D]; off += D
            w_Wrs = t_w[:, off:off + D]; off += D
        if cfg["head"]:
            w_WhT = t_w[:, off:off + 8]; off += 8
        w_iota = t_w[:, off:off + S]; off += S
        w_bb = t_w[:, off:off + 1]; off += 1
        w_bs = t_w[:, off:off + 1]; off += 1
        w_bh = t_w[:, off:off + 1]; off += 1

        def aggregate(tname, g, wbase):
            """Aggregate one group of `tname` into a PSUM tile; returns
            (sbuf m^T tile [P, ncols], ncols)."""
            t = types[tname]
            d_idx, d_rel, d_ivc = d_et[tname]
            wins = t["groups"][g]
            Wg = len(wins)
            ncols = wins[-1][0] + wins[-1][1]
            t_idx = mpool.tile([P, Wg], i32, tag=f"idx")
            nc.sync.dma_start(t_idx[:], d_idx[:, wbase:wbase + Wg])
            t_rel = mpool.tile([P, Wg], f32, tag=f"rel")
            nc.sync.dma_start(t_rel[:], d_rel[:, wbase:wbase + Wg])
            t_ivc = mpool.tile([P, Wg], f32, tag=f"ivc")
            nc.sync.dma_start(t_ivc[:], d_ivc[:, wbase:wbase + Wg])
            t_sel = mpool.tile([P, Wg * S], f32, tag="sel")
            sel3 = t_sel[:].rearrange("p (w s) -> p w s", w=Wg)
            nc.vector.tensor_tensor(
                out=sel3, in0=t_rel[:, :, None].to_broadcast([P, Wg, S]),
                in1=w_iota[:, None, :].to_broadcast([P, Wg, S]),
                op=mybir.AluOpType.is_equal)
            nc.vector.tensor_tensor(
                out=sel3, in0=sel3,
                in1=t_ivc[:, :, None].to_broadcast([P, Wg, S]),
                op=mybir.AluOpType.mult)
            t_ps = appool.tile([P, GROUP], f32, space="PSUM", tag="agg")
            for w, (coff, span) in enumerate(wins):
                t_g = gpool.tile([P, D], f32, tag="gw")
                nc.gpsimd.indirect_dma_start(
                    out=t_g[:], out_offset=None, in_=d_tab[t["tab"]][:],
                    in_offset=bass.IndirectOffsetOnAxis(
                        ap=t_idx[:, w:w + 1], axis=0))
                nc.tensor.matmul(
                    t_ps[:, coff:coff + span], lhsT=t_g[:],
                    rhs=t_sel[:, w * S:w * S + span],
                    start=(w == 0), stop=(w == Wg - 1))
            t_m = spool.tile([P, GROUP], f32, tag="mT")
            nc.vector.tensor_copy(out=t_m[:, :ncols], in_=t_ps[:, :ncols])
            return t_m, ncols

        # ---- b-node groups
        ngb = len(types["bb"]["groups"])
        ngs_on_b = len(types["sb"]["groups"])
        wb_bb = 0
        wb_sb = 0
        for g in range(ngb):
            m_bb, ncols = aggregate("bb", g, wb_bb)
            wb_bb += len(types["bb"]["groups"][g])
            has_sb = g < ngs_on_b
            if has_sb:
                m_sb, ncols_sb = aggregate("sb", g, wb_sb)
                wb_sb += len(types["sb"]["groups"][g])
            t_x = spool.tile([P, GROUP], f32, tag="xg")
            nc.sync.dma_start(t_x[:, :ncols],
                              d_xbT[:, g * GROUP:g * GROUP + ncols])
            ps2 = s2pool.tile([P, GROUP], f32, space="PSUM", tag="s2")
            nc.tensor.matmul(ps2[:, :ncols], lhsT=w_Wlbb, rhs=m_bb[:, :ncols],
                             start=True, stop=False)
            if has_sb:
                nc.tensor.matmul(ps2[:, :ncols_sb], lhsT=w_Wlsb,
                                 rhs=m_sb[:, :ncols_sb],
                                 start=False, stop=False)
            nc.tensor.matmul(ps2[:, :ncols], lhsT=w_Wrb, rhs=t_x[:, :ncols],
                             start=False, stop=True)
            t_o = spool.tile([P, GROUP], f32, tag="ob")
            nc.scalar.activation(out=t_o[:, :ncols], in_=ps2[:, :ncols],
                                 func=mybir.ActivationFunctionType.Lrelu,
                                 bias=w_bb, alpha=0.01)
            nc.sync.dma_start(d_nbT[:, g * GROUP:g * GROUP + ncols],
                              t_o[:, :ncols])
            if cfg["head"]:
                ps3 = hpool.tile([8, GROUP], f32, space="PSUM", tag="hd")
                nc.tensor.matmul(ps3[:, :ncols], lhsT=w_WhT,
                                 rhs=t_o[:, :ncols], start=True, stop=True)
                t_y = spool.tile([8, GROUP], f32, tag="yt")
                nc.vector.tensor_scalar_add(t_y[:, :ncols], ps3[:, :ncols],
                                            w_bh[:8])
                nc.sync.dma_start(d_yT[:, g * GROUP:g * GROUP + ncols],
                                  t_y[:, :ncols])

        # ---- s-node groups (layer 1 only)
        if cfg["out_s"]:
            wb_bs = 0
            for g in range(len(types["bs"]["groups"])):
                m_bs, ncols = aggregate("bs", g, wb_bs)
                wb_bs += len(types["bs"]["groups"][g])
                t_x = spool.tile([P, GROUP], f32, tag="xg")
                nc.sync.dma_start(t_x[:, :ncols],
                                  d_xsT[:, g * GROUP:g * GROUP + ncols])
                ps2 = s2pool.tile([P, GROUP], f32, space="PSUM", tag="s2")
                nc.tensor.matmul(ps2[:, :ncols], lhsT=w_Wlbs,
                                 rhs=m_bs[:, :ncols], start=True, stop=False)
                nc.tensor.matmul(ps2[:, :ncols], lhsT=w_Wrs,
                                 rhs=t_x[:, :ncols], start=False, stop=True)
                t_o = spool.tile([P, GROUP], f32, tag="ob")
                nc.scalar.activation(out=t_o[:, :ncols], in_=ps2[:, :ncols],
                                     func=mybir.ActivationFunctionType.Lrelu,
                                     bias=w_bs, alpha=0.01)
                nc.sync.dma_start(d_nsT[:, g * GROUP:g * GROUP + ncols],
                                  t_o[:, :ncols])

    nc.compile()
    return nc


def _pack_weights(cfg, Wlbb, Wlsb, Wrb, bb, bs_bias=None, Wlbs=None, Wrs=None,
                  WhT=None, bh0=None):
    nw = 3 * D + (2 * D if cfg["out_s"] else 0) + (8 if cfg["head"] else 0) + S + 3
    w = np.zeros((P, nw), np.float32)
    off = 0
    for M in [Wlbb, Wlsb, Wrb]:
        w[:, off:off + D] = M; off += D
    if cfg["out_s"]:
        w[:, off:off + D] = Wlbs; off += D
        w[:, off:off + D] = Wrs; off += D
    if cfg["head"]:
        w[:, off:off + 8] = WhT; off += 8
    w[:, off:off + S] = np.arange(S, dtype=np.float32)[None, :]; off += S
    w[:, off] = bb; off += 1
    if bs_bias is not None:
        w[:, off] = bs_bias
    off += 1
    if bh0 is not None:
        w[:8, off] = bh0
    return w


def kernel(x_b, x_s, Wl, bl, Wr, Wh, bh, ei_bb, ei_sb, ei_bs):
    x_b = np.asarray(x_b, np.float32); x_s = np.asarray(x_s, np.float32)
    Wl = np.asarray(Wl, np.float32); bl = np.asarray(bl, np.float32)
    Wr = np.asarray(Wr, np.float32); Wh = np.asarray(Wh, np.float32)
    bh = np.asarray(bh, np.float32)
    ei_bb = np.asarray(ei_bb); ei_sb = np.asarray(ei_sb); ei_bs = np.asarray(ei_bs)

    # ---------------- layer 1 prep (original node ids as gather indices)
    pc_bb = _shard_edges(ei_bb[0], ei_bb[1], NB)
    pc_sb = _shard_edges(ei_sb[0], ei_sb[1], NB)   # dst are b-nodes < NS
    pc_bs = _shard_edges(ei_bs[0], ei_bs[1], NS)
    i_bb, r_bb, v_bb, g_bb = _pack_type(pc_bb, NLB)
    i_sb, r_sb, v_sb, g_sb = _pack_type(pc_sb, NS // NCORES)
    i_bs, r_bs, v_bs, g_bs = _pack_type(pc_bs, NLS)

    cfgA = {
        "tabs": {"tab_b": NB, "tab_s": NS},
        "types": [
            {"name": "bb", "tab": "tab_b", "Wtot": i_bb.shape[2], "groups": g_bb},
            {"name": "sb", "tab": "tab_s", "Wtot": i_sb.shape[2], "groups": g_sb},
            {"name": "bs", "tab": "tab_b", "Wtot": i_bs.shape[2], "groups": g_bs},
        ],
        "head": False, "out_s": True,
    }
    ncA = _build_launch(cfgA)
    wA = _pack_weights(cfgA, Wl[0, 0], Wl[0, 1], Wr[0, 0] + Wr[0, 1],
                       bl[0, 0] + bl[0, 1], bs_bias=bl[0, 2],
                       Wlbs=Wl[0, 2], Wrs=Wr[0, 2])
    in_maps = []
    for c in range(NCORES):
        in_maps.append({
            "tab_b": x_b, "tab_s": x_s,
            "xbT": np.ascontiguousarray(x_b[c::NCORES].T),
            "xsT": np.ascontiguousarray(x_s[c::NCORES].T),
            "wts": wA,
            "idx_bb": i_bb[c], "rel_bb": r_bb[c], "ivc_bb": v_bb[c],
            "idx_sb": i_sb[c], "rel_sb": r_sb[c], "ivc_sb": v_sb[c],
            "idx_bs": i_bs[c], "rel_bs": r_bs[c], "ivc_bs": v_bs[c],
        })
    resA = run_bass_kernel_spmd(ncA, in_maps, core_ids=list(range(NCORES)))
    nbT = [resA.results[c]["nbT"] for c in range(NCORES)]
    nsT = [resA.results[c]["nsT"] for c in range(NCORES)]

    # ---------------- layer 2: host halo exchange + index translation
    xb1 = np.concatenate([t.T for t in nbT], 0)   # [NB, D] core-block order
    xs1 = np.concatenate([t.T for t in nsT], 0)   # [NS, D]

    def tr_b(v):
        return (v % NCORES) * NLB + v // NCORES

    def tr_s(v):
        return (v % NCORES) * NLS + v // NCORES

    pc_bb2 = _shard_edges(tr_b(ei_bb[0]), ei_bb[1], NB)
    pc_sb2 = _shard_edges(tr_s(ei_sb[0]), ei_sb[1], NB)
    i_bb2, r_bb2, v_bb2, g_bb2 = _pack_type(pc_bb2, NLB)
    i_sb2, r_sb2, v_sb2, g_sb2 = _pack_type(pc_sb2, NS // NCORES)

    cfgB = {
        "tabs": {"tab_b": NB, "tab_s": NS},
        "types": [
            {"name": "bb", "tab": "tab_b", "Wtot": i_bb2.shape[2], "groups": g_bb2},
            {"name": "sb", "tab": "tab_s", "Wtot": i_sb2.shape[2], "groups": g_sb2},
        ],
        "head": True, "out_s": False,
    }
    ncB = _build_launch(cfgB)
    wB = _pack_weights(cfgB, Wl[1, 0], Wl[1, 1], Wr[1, 0] + Wr[1, 1],
                       bl[1, 0] + bl[1, 1], WhT=Wh.T, bh0=bh)
    in_mapsB = []
    for c in range(NCORES):
        in_mapsB.append({
            "tab_b": xb1, "tab_s": xs1,
            "xbT": nbT[c], "wts": wB,
            "idx_bb": i_bb2[c], "rel_bb": r_bb2[c], "ivc_bb": v_bb2[c],
            "idx_sb": i_sb2[c], "rel_sb": r_sb2[c], "ivc_sb": v_sb2[c],
        })
    resB = run_bass_kernel_spmd(ncB, in_mapsB, core_ids=list(range(NCORES)))

    y = np.empty((NB, 8), np.float32)
    for c in range(NCORES):
        y[np.arange(NLB) * NCORES + c] = resB.results[c]["yT"].T
    return y


# revision 4
# speedup vs baseline: 1.8222x; 1.8222x over previous
"""HGNN (2-layer hetero GraphSAGE + 8 heads) on 8 trn2 NeuronCores.

Sharding: dst-node interleaved (core = v % 8, local = v // 8). Each layer is
one SPMD NEFF launch; the host performs the inter-layer halo exchange by
concatenating per-core outputs into fresh gather tables (indices are
pre-translated into the concatenated layout).

Device-side per layer, per core:
  - For each 512-dst-column PSUM group, edges (sorted by dst) are cut into
    128-edge windows on a column grid that is uniform across cores
    (min-over-cores advance), so a single program serves all 8 cores.
  - Per window: one indirect DMA gathers the 128 source rows [128, 128];
    a selection matrix sel[e, j] = (rel_dst[e] == j) * invcnt[e] is built
    with two batched DVE ops; PE accumulates g.T @ sel into the PSUM group,
    yielding the transposed scatter-mean m^T [128 feat, 512 dst] directly.
  - Dense stage: nb^T = Wl_bb.T @ m_bb^T + Wl_sb.T @ m_sb^T + Wr.T @ x^T,
    then bias + leaky-relu fused on the scalar engine. Head (layer 2) is one
    more matmul with Wh^T producing y^T [8, dst].
"""
import os
import numpy as np

import concourse.bass as bass
import concourse.bacc as bacc
import concourse.mybir as mybir
import concourse.tile as tile
from concourse.bass_utils import run_bass_kernel_spmd

P = 128
D = 128
NCORES = 8
GROUP = 512       # psum columns per accumulation group
S = 32            # max dst-column span per 128-edge window
NB, NS = 100000, 50000
NLB, NLS = NB // NCORES, NS // NCORES   # 12500, 6250


# ---------------------------------------------------------------- host prep
def _shard_edges(src, dst, n_dst):
    """Split edges by dst core; per core return (src, dst_local) dst-sorted."""
    core = dst % NCORES
    loc = dst // NCORES
    out = []
    for c in range(NCORES):
        m = core == c
        s, d = src[m], loc[m]
        o = np.argsort(d, kind="stable")
        out.append((s[o].astype(np.int64), d[o].astype(np.int64)))
    return out


def _pack_type(per_core, n_loc):
    """Uniform-across-cores window packing.

    Returns (idx [NCORES,128,Wtot] i32, rel [NCORES,128,Wtot] f32,
             invc [NCORES,128,Wtot] f32, groups: list of list[(col_off, span)]).
    """
    ngroups = (n_loc + GROUP - 1) // GROUP
    # per-core prefix counts over columns
    counts = []
    for s, d in per_core:
        counts.append(np.bincount(d, minlength=n_loc))
    cum = [np.concatenate([[0], np.cumsum(c)]) for c in counts]  # [n_loc+1]
    invc_dst = [1.0 / np.maximum(c, 1) for c in counts]

    groups = []
    idx_cols, rel_cols, invc_cols = (
        [[] for _ in range(NCORES)],
        [[] for _ in range(NCORES)],
        [[] for _ in range(NCORES)],
    )
    for g in range(ngroups):
        c0, c1 = g * GROUP, min((g + 1) * GROUP, n_loc)
        wins = []
        c = c0
        while c < c1:
            span = min(S, c1 - c)
            # shrink span until every core has <= 128 edges in [c, c+span)
            while span > 1:
                ok = True
                for cc in range(NCORES):
                    if cum[cc][c + span] - cum[cc][c] > P:
                        ok = False
                        break
                if ok:
                    break
                span -= 1
            wins.append((c - c0, span))
            for cc in range(NCORES):
                s_arr, d_arr = per_core[cc]
                a, b = cum[cc][c], cum[cc][c + span]
                n = b - a
                assert n <= P, "single column exceeded 128 edges"
                icol = np.zeros(P, np.int32)
                rcol = np.full(P, -1.0, np.float32)
                vcol = np.zeros(P, np.float32)
                icol[:n] = s_arr[a:b]
                rcol[:n] = (d_arr[a:b] - c).astype(np.float32)
                vcol[:n] = invc_dst[cc][d_arr[a:b]].astype(np.float32)
                idx_cols[cc].append(icol)
                rel_cols[cc].append(rcol)
                invc_cols[cc].append(vcol)
            c += span
        groups.append(wins)

    idx = np.stack([np.stack(cols, 1) for cols in idx_cols])     # [NC, P, Wtot]
    rel = np.stack([np.stack(cols, 1) for cols in rel_cols])
    invc = np.stack([np.stack(cols, 1) for cols in invc_cols])
    return idx.astype(np.int32), rel.astype(np.float32), invc.astype(np.float32), groups


# ------------------------------------------------------------- device build
def _build_launch(cfg):
    """Build one layer's SPMD program. cfg keys:
      tabs: {name: nrows} gather tables
      types: list of dicts(name, tab, Wtot, groups, n_loc)
      head: bool — add 8-head output (layer 2)
      out_s: bool — emit s-node output (layer 1)
    """
    nc = bacc.Bacc("TRN2", target_bir_lowering=False, debug=False,
                   num_devices=NCORES)
    f32, i32 = mybir.dt.float32, mybir.dt.int32

    d_tab = {k: nc.dram_tensor(k, [n, D], f32, kind="ExternalInput")
             for k, n in cfg["tabs"].items()}
    d_xbT = nc.dram_tensor("xbT", [P, NLB], f32, kind="ExternalInput")
    d_xsT = (nc.dram_tensor("xsT", [P, NLS], f32, kind="ExternalInput")
             if cfg["out_s"] else None)
    # packed weights: Wl_bb | Wl_sb | Wr_b | [Wl_bs | Wr_s] | WhT | iota | biases
    nw = 3 * D + (2 * D if cfg["out_s"] else 0) + (8 if cfg["head"] else 0) + S + 3
    d_w = nc.dram_tensor("wts", [P, nw], f32, kind="ExternalInput")
    d_et = {}
    for t in cfg["types"]:
        W = t["Wtot"]
        d_et[t["name"]] = (
            nc.dram_tensor(f'idx_{t["name"]}', [P, W], i32, kind="ExternalInput"),
            nc.dram_tensor(f'rel_{t["name"]}', [P, W], f32, kind="ExternalInput"),
            nc.dram_tensor(f'ivc_{t["name"]}', [P, W], f32, kind="ExternalInput"),
        )
    d_nbT = nc.dram_tensor("nbT", [P, NLB], f32, kind="ExternalOutput")
    d_nsT = (nc.dram_tensor("nsT", [P, NLS], f32, kind="ExternalOutput")
             if cfg["out_s"] else None)
    d_yT = (nc.dram_tensor("yT", [8, NLB], f32, kind="ExternalOutput")
            if cfg["head"] else None)

    types = {t["name"]: t for t in cfg["types"]}

    from contextlib import ExitStack
    with tile.TileContext(nc) as tc, ExitStack() as ctx:
        wpool = ctx.enter_context(tc.tile_pool(name="w", bufs=1))
        gpool = ctx.enter_context(tc.tile_pool(name="g", bufs=12))
        mpool = ctx.enter_context(tc.tile_pool(name="m", bufs=3))
        spool = ctx.enter_context(tc.tile_pool(name="s", bufs=3))
        appool = ctx.enter_context(tc.tile_pool(name="ap", bufs=3, space="PSUM"))
        s2pool = ctx.enter_context(tc.tile_pool(name="s2", bufs=2, space="PSUM"))
        hpool = (ctx.enter_context(tc.tile_pool(name="h", bufs=2, space="PSUM"))
                 if cfg["head"] else None)

        t_w = wpool.tile([P, nw], f32)
        nc.sync.dma_start(t_w[:], d_w[:])
        off = 0
        w_Wlbb = t_w[:, off:off + D]; off += D
        w_Wlsb = t_w[:, off:off + D]; off += D
        w_Wrb = t_w[:, off:off + D]; off += D
        if cfg["out_s"]:
            w_Wlbs = t_w[:, off:off + D]; off += D
            w_Wrs = t_w[:, off:off + D]; off += D
        if cfg["head"]:
            w_WhT = t_w[:, off:off + 8]; off += 8
        w_iota = t_w[:, off:off + S]; off += S
        w_bb = t_w[:, off:off + 1]; off += 1
        w_bs = t_w[:, off:off + 1]; off += 1
        w_bh = t_w[:, off:off + 1]; off += 1

        def aggregate(tname, g, wbase):
            """Aggregate one group of `tname` into a PSUM tile; returns
            (sbuf m^T tile [P, ncols], ncols)."""
            t = types[tname]
            d_idx, d_rel, d_ivc = d_et[tname]
            wins = t["groups"][g]
            Wg = len(wins)
            ncols = wins[-1][0] + wins[-1][1]
            t_idx = mpool.tile([P, Wg], i32, tag=f"idx")
            nc.sync.dma_start(t_idx[:], d_idx[:, wbase:wbase + Wg])
            t_rel = mpool.tile([P, Wg], f32, tag=f"rel")
            nc.sync.dma_start(t_rel[:], d_rel[:, wbase:wbase + Wg])
            t_ivc = mpool.tile([P, Wg], f32, tag=f"ivc")
            nc.sync.dma_start(t_ivc[:], d_ivc[:, wbase:wbase + Wg])
            t_sel = mpool.tile([P, Wg * S], f32, tag="sel")
            sel3 = t_sel[:].rearrange("p (w s) -> p w s", w=Wg)
            nc.vector.tensor_tensor(
                out=sel3, in0=t_rel[:, :, None].to_broadcast([P, Wg, S]),
                in1=w_iota[:, None, :].to_broadcast([P, Wg, S]),
                op=mybir.AluOpType.is_equal)
            nc.vector.tensor_tensor(
                out=sel3, in0=sel3,
                in1=t_ivc[:, :, None].to_broadcast([P, Wg, S]),
                op=mybir.AluOpType.mult)
            t_ps = appool.tile([P, GROUP], f32, space="PSUM", tag="agg")
            for w, (coff, span) in enumerate(wins):
                t_g = gpool.tile([P, D], f32, tag="gw")
                nc.gpsimd.indirect_dma_start(
                    out=t_g[:], out_offset=None, in_=d_tab[t["tab"]][:],
                    in_offset=bass.IndirectOffsetOnAxis(
                        ap=t_idx[:, w:w + 1], axis=0))
                nc.tensor.matmul(
                    t_ps[:, coff:coff + span], lhsT=t_g[:],
                    rhs=t_sel[:, w * S:w * S + span],
                    start=(w == 0), stop=(w == Wg - 1))
            t_m = spool.tile([P, GROUP], f32, tag="mT")
            nc.vector.tensor_copy(out=t_m[:, :ncols], in_=t_ps[:, :ncols])
            return t_m, ncols

        # ---- b-node groups
        ngb = len(types["bb"]["groups"])
        ngs_on_b = len(types["sb"]["groups"])
        wb_bb = 0
        wb_sb = 0
        for g in range(ngb):
            m_bb, ncols = aggregate("bb", g, wb_bb)
            wb_bb += len(types["bb"]["groups"][g])
            has_sb = g < ngs_on_b
            if has_sb:
                m_sb, ncols_sb = aggregate("sb", g, wb_sb)
                wb_sb += len(types["sb"]["groups"][g])
            t_x = spool.tile([P, GROUP], f32, tag="xg")
            nc.sync.dma_start(t_x[:, :ncols],
                              d_xbT[:, g * GROUP:g * GROUP + ncols])
            ps2 = s2pool.tile([P, GROUP], f32, space="PSUM", tag="s2")
            nc.tensor.matmul(ps2[:, :ncols], lhsT=w_Wlbb, rhs=m_bb[:, :ncols],
                             start=True, stop=False)
            if has_sb:
                nc.tensor.matmul(ps2[:, :ncols_sb], lhsT=w_Wlsb,
                                 rhs=m_sb[:, :ncols_sb],
                                 start=False, stop=False)
            nc.tensor.matmul(ps2[:, :ncols], lhsT=w_Wrb, rhs=t_x[:, :ncols],
                             start=False, stop=True)
            t_o = spool.tile([P, GROUP], f32, tag="ob")
            nc.scalar.activation(out=t_o[:, :ncols], in_=ps2[:, :ncols],
                                 func=mybir.ActivationFunctionType.Lrelu,
                                 bias=w_bb, alpha=0.01)
            nc.sync.dma_start(d_nbT[:, g * GROUP:g * GROUP + ncols],
                              t_o[:, :ncols])
            if cfg["head"]:
                ps3 = hpool.tile([8, GROUP], f32, space="PSUM", tag="hd")
                nc.tensor.matmul(ps3[:, :ncols], lhsT=w_WhT,
                                 rhs=t_o[:, :ncols], start=True, stop=True)
                t_y = spool.tile([8, GROUP], f32, tag="yt")
                nc.vector.tensor_scalar_add(t_y[:, :ncols], ps3[:, :ncols],
                                            w_bh[:8])
                nc.sync.dma_start(d_yT[:, g * GROUP:g * GROUP + ncols],
                                  t_y[:, :ncols])

        # ---- s-node groups (layer 1 only)
        if cfg["out_s"]:
            wb_bs = 0
            for g in range(len(types["bs"]["groups"])):
                m_bs, ncols = aggregate("bs", g, wb_bs)
                wb_bs += len(types["bs"]["groups"][g])
                t_x = spool.tile([P, GROUP], f32, tag="xg")
                nc.sync.dma_start(t_x[:, :ncols],
                                  d_xsT[:, g * GROUP:g * GROUP + ncols])
                ps2 = s2pool.tile([P, GROUP], f32, space="PSUM", tag="s2")
                nc.tensor.matmul(ps2[:, :ncols], lhsT=w_Wlbs,
                                 rhs=m_bs[:, :ncols], start=True, stop=False)
                nc.tensor.matmul(ps2[:, :ncols], lhsT=w_Wrs,
                                 rhs=t_x[:, :ncols], start=False, stop=True)
                t_o = spool.tile([P, GROUP], f32, tag="ob")
                nc.scalar.activation(out=t_o[:, :ncols], in_=ps2[:, :ncols],
                                     func=mybir.ActivationFunctionType.Lrelu,
                                     bias=w_bs, alpha=0.01)
                nc.sync.dma_start(d_nsT[:, g * GROUP:g * GROUP + ncols],
                                  t_o[:, :ncols])

    nc.compile()
    return nc


def _pack_weights(cfg, Wlbb, Wlsb, Wrb, bb, bs_bias=None, Wlbs=None, Wrs=None,
                  WhT=None, bh0=None):
    nw = 3 * D + (2 * D if cfg["out_s"] else 0) + (8 if cfg["head"] else 0) + S + 3
    w = np.zeros((P, nw), np.float32)
    off = 0
    for M in [Wlbb, Wlsb, Wrb]:
        w[:, off:off + D] = M; off += D
    if cfg["out_s"]:
        w[:, off:off + D] = Wlbs; off += D
        w[:, off:off + D] = Wrs; off += D
    if cfg["head"]:
        w[:, off:off + 8] = WhT; off += 8
    w[:, off:off + S] = np.arange(S, dtype=np.float32)[None, :]; off += S
    w[:, off] = bb; off += 1
    if bs_bias is not None:
        w[:, off] = bs_bias
    off += 1
    if bh0 is not None:
        w[:8, off] = bh0
    return w


LAST_HW_NS = None


def kernel(x_b, x_s, Wl, bl, Wr, Wh, bh, ei_bb, ei_sb, ei_bs):
    x_b = np.asarray(x_b, np.float32); x_s = np.asarray(x_s, np.float32)
    Wl = np.asarray(Wl, np.float32); bl = np.asarray(bl, np.float32)
    Wr = np.asarray(Wr, np.float32); Wh = np.asarray(Wh, np.float32)
    bh = np.asarray(bh, np.float32)
    ei_bb = np.asarray(ei_bb); ei_sb = np.asarray(ei_sb); ei_bs = np.asarray(ei_bs)

    # ---------------- layer 1 prep (original node ids as gather indices)
    pc_bb = _shard_edges(ei_bb[0], ei_bb[1], NB)
    pc_sb = _shard_edges(ei_sb[0], ei_sb[1], NB)   # dst are b-nodes < NS
    pc_bs = _shard_edges(ei_bs[0], ei_bs[1], NS)
    i_bb, r_bb, v_bb, g_bb = _pack_type(pc_bb, NLB)
    i_sb, r_sb, v_sb, g_sb = _pack_type(pc_sb, NS // NCORES)
    i_bs, r_bs, v_bs, g_bs = _pack_type(pc_bs, NLS)

    cfgA = {
        "tabs": {"tab_b": NB, "tab_s": NS},
        "types": [
            {"name": "bb", "tab": "tab_b", "Wtot": i_bb.shape[2], "groups": g_bb},
            {"name": "sb", "tab": "tab_s", "Wtot": i_sb.shape[2], "groups": g_sb},
            {"name": "bs", "tab": "tab_b", "Wtot": i_bs.shape[2], "groups": g_bs},
        ],
        "head": False, "out_s": True,
    }
    ncA = _build_launch(cfgA)
    wA = _pack_weights(cfgA, Wl[0, 0], Wl[0, 1], Wr[0, 0] + Wr[0, 1],
                       bl[0, 0] + bl[0, 1], bs_bias=bl[0, 2],
                       Wlbs=Wl[0, 2], Wrs=Wr[0, 2])
    in_maps = []
    for c in range(NCORES):
        in_maps.append({
            "tab_b": x_b, "tab_s": x_s,
            "xbT": np.ascontiguousarray(x_b[c::NCORES].T),
            "xsT": np.ascontiguousarray(x_s[c::NCORES].T),
            "wts": wA,
            "idx_bb": i_bb[c], "rel_bb": r_bb[c], "ivc_bb": v_bb[c],
            "idx_sb": i_sb[c], "rel_sb": r_sb[c], "ivc_sb": v_sb[c],
            "idx_bs": i_bs[c], "rel_bs": r_bs[c], "ivc_bs": v_bs[c],
        })
    _tr = False
    resA = run_bass_kernel_spmd(ncA, in_maps, core_ids=list(range(NCORES)),
                                trace=_tr, trace_cores=[0] if _tr else None)
    if _tr:
        print("launchA exec_ns:", resA.exec_time_ns,
              "trace:", (resA.instructions_and_trace or (None, None))[1], flush=True)
    nbT = [resA.results[c]["nbT"] for c in range(NCORES)]
    nsT = [resA.results[c]["nsT"] for c in range(NCORES)]

    # ---------------- layer 2: host halo exchange + index translation
    xb1 = np.concatenate([t.T for t in nbT], 0)   # [NB, D] core-block order
    xs1 = np.concatenate([t.T for t in nsT], 0)   # [NS, D]

    def tr_b(v):
        return (v % NCORES) * NLB + v // NCORES

    def tr_s(v):
        return (v % NCORES) * NLS + v // NCORES

    pc_bb2 = _shard_edges(tr_b(ei_bb[0]), ei_bb[1], NB)
    pc_sb2 = _shard_edges(tr_s(ei_sb[0]), ei_sb[1], NB)
    i_bb2, r_bb2, v_bb2, g_bb2 = _pack_type(pc_bb2, NLB)
    i_sb2, r_sb2, v_sb2, g_sb2 = _pack_type(pc_sb2, NS // NCORES)

    cfgB = {
        "tabs": {"tab_b": NB, "tab_s": NS},
        "types": [
            {"name": "bb", "tab": "tab_b", "Wtot": i_bb2.shape[2], "groups": g_bb2},
            {"name": "sb", "tab": "tab_s", "Wtot": i_sb2.shape[2], "groups": g_sb2},
        ],
        "head": True, "out_s": False,
    }
    ncB = _build_launch(cfgB)
    wB = _pack_weights(cfgB, Wl[1, 0], Wl[1, 1], Wr[1, 0] + Wr[1, 1],
                       bl[1, 0] + bl[1, 1], WhT=Wh.T, bh0=bh)
    in_mapsB = []
    for c in range(NCORES):
        in_mapsB.append({
            "tab_b": xb1, "tab_s": xs1,
            "xbT": nbT[c], "wts": wB,
            "idx_bb": i_bb2[c], "rel_bb": r_bb2[c], "ivc_bb": v_bb2[c],
            "idx_sb": i_sb2[c], "rel_sb": r_sb2[c], "ivc_sb": v_sb2[c],
        })
    resB = run_bass_kernel_spmd(ncB, in_mapsB, core_ids=list(range(NCORES)),
                                trace=_tr, trace_cores=[0] if _tr else None)
    if _tr:
        print("launchB exec_ns:", resB.exec_time_ns,
              "trace:", (resB.instructions_and_trace or (None, None))[1], flush=True)
    global LAST_HW_NS
    if resA.exec_time_ns and resB.exec_time_ns:
        LAST_HW_NS = int(resA.exec_time_ns) + int(resB.exec_time_ns)

    y = np.empty((NB, 8), np.float32)
    for c in range(NCORES):
        y[np.arange(NLB) * NCORES + c] = resB.results[c]["yT"].T
    return y


# revision 5
# speedup vs baseline: 2.0559x; 1.1282x over previous
"""HGNN (2-layer hetero GraphSAGE + 8 heads) on 8 trn2 NeuronCores.

Sharding: dst-node interleaved (core = v % 8, local = v // 8). Each layer is
one SPMD NEFF launch; the host performs the inter-layer halo exchange by
concatenating per-core outputs into fresh gather tables (indices are
pre-translated into the concatenated layout).

Device-side per layer, per core:
  - For each 512-dst-column PSUM group, edges (sorted by dst) are cut into
    128-edge windows on a column grid that is uniform across cores
    (min-over-cores advance), so a single program serves all 8 cores.
  - Per window: one indirect DMA gathers the 128 source rows [128, 128];
    a selection matrix sel[e, j] = (rel_dst[e] == j) * invcnt[e] is built
    with two batched DVE ops; PE accumulates g.T @ sel into the PSUM group,
    yielding the transposed scatter-mean m^T [128 feat, 512 dst] directly.
  - Dense stage: nb^T = Wl_bb.T @ m_bb^T + Wl_sb.T @ m_sb^T + Wr.T @ x^T,
    then bias + leaky-relu fused on the scalar engine. Head (layer 2) is one
    more matmul with Wh^T producing y^T [8, dst].
"""
import os
import time
import numpy as np

import concourse.bass as bass
import concourse.bacc as bacc
import concourse.mybir as mybir
import concourse.tile as tile
from concourse.bass_utils import run_bass_kernel_spmd

P = 128
D = 128
NCORES = 8
GROUP = 512       # psum columns per accumulation group
S = 32            # max dst-column span per 128-edge window
NB, NS = 100000, 50000
NLB, NLS = NB // NCORES, NS // NCORES   # 12500, 6250


# ---------------------------------------------------------------- host prep
def _shard_edges(src, dst, n_dst):
    """Split edges by dst core; per core return (src, dst_local) dst-sorted."""
    core = dst % NCORES
    loc = dst // NCORES
    out = []
    for c in range(NCORES):
        m = core == c
        s, d = src[m], loc[m]
        o = np.argsort(d, kind="stable")
        out.append((s[o].astype(np.int64), d[o].astype(np.int64)))
    return out


def _pack_type(per_core, n_loc):
    """Uniform-across-cores window packing.

    Returns (idx [NCORES,128,Wtot] i32, rel [NCORES,128,Wtot] f32,
             invc [NCORES,128,Wtot] f32, groups: list of list[(col_off, span)]).
    """
    ngroups = (n_loc + GROUP - 1) // GROUP
    # per-core prefix counts over columns
    counts = []
    for s, d in per_core:
        counts.append(np.bincount(d, minlength=n_loc))
    cum = [np.concatenate([[0], np.cumsum(c)]) for c in counts]  # [n_loc+1]
    invc_dst = [1.0 / np.maximum(c, 1) for c in counts]

    groups = []
    idx_cols, rel_cols, invc_cols = (
        [[] for _ in range(NCORES)],
        [[] for _ in range(NCORES)],
        [[] for _ in range(NCORES)],
    )
    for g in range(ngroups):
        c0, c1 = g * GROUP, min((g + 1) * GROUP, n_loc)
        wins = []
        c = c0
        while c < c1:
            span = min(S, c1 - c)
            # shrink span until every core has <= 128 edges in [c, c+span)
            while span > 1:
                ok = True
                for cc in range(NCORES):
                    if cum[cc][c + span] - cum[cc][c] > P:
                        ok = False
                        break
                if ok:
                    break
                span -= 1
            wins.append((c - c0, span))
            for cc in range(NCORES):
                s_arr, d_arr = per_core[cc]
                a, b = cum[cc][c], cum[cc][c + span]
                n = b - a
                assert n <= P, "single column exceeded 128 edges"
                icol = np.zeros(P, np.int32)
                rcol = np.full(P, -1.0, np.float32)
                vcol = np.zeros(P, np.float32)
                icol[:n] = s_arr[a:b]
                rcol[:n] = (d_arr[a:b] - c).astype(np.float32)
                vcol[:n] = invc_dst[cc][d_arr[a:b]].astype(np.float32)
                idx_cols[cc].append(icol)
                rel_cols[cc].append(rcol)
                invc_cols[cc].append(vcol)
            c += span
        groups.append(wins)

    idx = np.stack([np.stack(cols, 1) for cols in idx_cols])     # [NC, P, Wtot]
    rel = np.stack([np.stack(cols, 1) for cols in rel_cols])
    invc = np.stack([np.stack(cols, 1) for cols in invc_cols])
    return idx.astype(np.int32), rel.astype(np.float32), invc.astype(np.float32), groups


# ------------------------------------------------------------- device build
def _build_launch(cfg):
    """Build one layer's SPMD program. cfg keys:
      tabs: {name: nrows} gather tables
      types: list of dicts(name, tab, Wtot, groups, n_loc)
      head: bool — add 8-head output (layer 2)
      out_s: bool — emit s-node output (layer 1)
    """
    nc = bacc.Bacc("TRN2", target_bir_lowering=False, debug=False,
                   num_devices=NCORES)
    f32, i32 = mybir.dt.float32, mybir.dt.int32

    d_tab = {k: nc.dram_tensor(k, [n, D], f32, kind="ExternalInput")
             for k, n in cfg["tabs"].items()}
    d_xbT = nc.dram_tensor("xbT", [P, NLB], f32, kind="ExternalInput")
    d_xsT = (nc.dram_tensor("xsT", [P, NLS], f32, kind="ExternalInput")
             if cfg["out_s"] else None)
    # packed weights: Wl_bb | Wl_sb | Wr_b | [Wl_bs | Wr_s] | WhT | iota | biases
    nw = 3 * D + (2 * D if cfg["out_s"] else 0) + (8 if cfg["head"] else 0) + S + 3
    d_w = nc.dram_tensor("wts", [P, nw], f32, kind="ExternalInput")
    d_et = {}
    for t in cfg["types"]:
        W = t["Wtot"]
        d_et[t["name"]] = (
            nc.dram_tensor(f'idx_{t["name"]}', [P, W], i32, kind="ExternalInput"),
            nc.dram_tensor(f'rel_{t["name"]}', [P, W], f32, kind="ExternalInput"),
            nc.dram_tensor(f'ivc_{t["name"]}', [P, W], f32, kind="ExternalInput"),
        )
    d_nbT = nc.dram_tensor("nbT", [P, NLB], f32, kind="ExternalOutput")
    d_nsT = (nc.dram_tensor("nsT", [P, NLS], f32, kind="ExternalOutput")
             if cfg["out_s"] else None)
    d_yT = (nc.dram_tensor("yT", [8, NLB], f32, kind="ExternalOutput")
            if cfg["head"] else None)

    types = {t["name"]: t for t in cfg["types"]}

    from contextlib import ExitStack
    with tile.TileContext(nc) as tc, ExitStack() as ctx:
        wpool = ctx.enter_context(tc.tile_pool(name="w", bufs=1))
        gpool = ctx.enter_context(tc.tile_pool(name="g", bufs=12))
        mpool = ctx.enter_context(tc.tile_pool(name="m", bufs=3))
        spool = ctx.enter_context(tc.tile_pool(name="s", bufs=3))
        appool = ctx.enter_context(tc.tile_pool(name="ap", bufs=3, space="PSUM"))
        s2pool = ctx.enter_context(tc.tile_pool(name="s2", bufs=2, space="PSUM"))
        hpool = (ctx.enter_context(tc.tile_pool(name="h", bufs=2, space="PSUM"))
                 if cfg["head"] else None)

        t_w = wpool.tile([P, nw], f32)
        nc.sync.dma_start(t_w[:], d_w[:])
        off = 0
        w_Wlbb = t_w[:, off:off + D]; off += D
        w_Wlsb = t_w[:, off:off + D]; off += D
        w_Wrb = t_w[:, off:off + D]; off += D
        if cfg["out_s"]:
            w_Wlbs = t_w[:, off:off + D]; off += D
            w_Wrs = t_w[:, off:off + D]; off += D
        if cfg["head"]:
            w_WhT = t_w[:, off:off + 8]; off += 8
        w_iota = t_w[:, off:off + S]; off += S
        w_bb = t_w[:, off:off + 1]; off += 1
        w_bs = t_w[:, off:off + 1]; off += 1
        w_bh = t_w[:, off:off + 1]; off += 1

        def aggregate(tname, g, wbase):
            """Aggregate one group of `tname` into a PSUM tile; returns
            (sbuf m^T tile [P, ncols], ncols)."""
            t = types[tname]
            d_idx, d_rel, d_ivc = d_et[tname]
            wins = t["groups"][g]
            Wg = len(wins)
            ncols = wins[-1][0] + wins[-1][1]
            t_idx = mpool.tile([P, Wg], i32, tag=f"idx")
            nc.sync.dma_start(t_idx[:], d_idx[:, wbase:wbase + Wg])
            t_rel = mpool.tile([P, Wg], f32, tag=f"rel")
            nc.sync.dma_start(t_rel[:], d_rel[:, wbase:wbase + Wg])
            t_ivc = mpool.tile([P, Wg], f32, tag=f"ivc")
            nc.sync.dma_start(t_ivc[:], d_ivc[:, wbase:wbase + Wg])
            t_sel = mpool.tile([P, Wg * S], f32, tag="sel")
            sel3 = t_sel[:].rearrange("p (w s) -> p w s", w=Wg)
            nc.vector.tensor_tensor(
                out=sel3, in0=t_rel[:, :, None].to_broadcast([P, Wg, S]),
                in1=w_iota[:, None, :].to_broadcast([P, Wg, S]),
                op=mybir.AluOpType.is_equal)
            nc.vector.tensor_tensor(
                out=sel3, in0=sel3,
                in1=t_ivc[:, :, None].to_broadcast([P, Wg, S]),
                op=mybir.AluOpType.mult)
            t_ps = appool.tile([P, GROUP], f32, space="PSUM", tag="agg")
            for w, (coff, span) in enumerate(wins):
                t_g = gpool.tile([P, D], f32, tag="gw")
                nc.gpsimd.indirect_dma_start(
                    out=t_g[:], out_offset=None, in_=d_tab[t["tab"]][:],
                    in_offset=bass.IndirectOffsetOnAxis(
                        ap=t_idx[:, w:w + 1], axis=0))
                nc.tensor.matmul(
                    t_ps[:, coff:coff + span], lhsT=t_g[:],
                    rhs=t_sel[:, w * S:w * S + span],
                    start=(w == 0), stop=(w == Wg - 1))
            t_m = spool.tile([P, GROUP], f32, tag="mT")
            nc.vector.tensor_copy(out=t_m[:, :ncols], in_=t_ps[:, :ncols])
            return t_m, ncols

        # ---- b-node groups
        ngb = len(types["bb"]["groups"])
        ngs_on_b = len(types["sb"]["groups"])
        wb_bb = 0
        wb_sb = 0
        for g in range(ngb):
            m_bb, ncols = aggregate("bb", g, wb_bb)
            wb_bb += len(types["bb"]["groups"][g])
            has_sb = g < ngs_on_b
            if has_sb:
                m_sb, ncols_sb = aggregate("sb", g, wb_sb)
                wb_sb += len(types["sb"]["groups"][g])
            t_x = spool.tile([P, GROUP], f32, tag="xg")
            nc.sync.dma_start(t_x[:, :ncols],
                              d_xbT[:, g * GROUP:g * GROUP + ncols])
            ps2 = s2pool.tile([P, GROUP], f32, space="PSUM", tag="s2")
            nc.tensor.matmul(ps2[:, :ncols], lhsT=w_Wlbb, rhs=m_bb[:, :ncols],
                             start=True, stop=False)
            if has_sb:
                nc.tensor.matmul(ps2[:, :ncols_sb], lhsT=w_Wlsb,
                                 rhs=m_sb[:, :ncols_sb],
                                 start=False, stop=False)
            nc.tensor.matmul(ps2[:, :ncols], lhsT=w_Wrb, rhs=t_x[:, :ncols],
                             start=False, stop=True)
            t_o = spool.tile([P, GROUP], f32, tag="ob")
            nc.scalar.activation(out=t_o[:, :ncols], in_=ps2[:, :ncols],
                                 func=mybir.ActivationFunctionType.Lrelu,
                                 bias=w_bb, alpha=0.01)
            nc.sync.dma_start(d_nbT[:, g * GROUP:g * GROUP + ncols],
                              t_o[:, :ncols])
            if cfg["head"]:
                ps3 = hpool.tile([8, GROUP], f32, space="PSUM", tag="hd")
                nc.tensor.matmul(ps3[:, :ncols], lhsT=w_WhT,
                                 rhs=t_o[:, :ncols], start=True, stop=True)
                t_y = spool.tile([8, GROUP], f32, tag="yt")
                nc.vector.tensor_scalar_add(t_y[:, :ncols], ps3[:, :ncols],
                                            w_bh[:8])
                nc.sync.dma_start(d_yT[:, g * GROUP:g * GROUP + ncols],
                                  t_y[:, :ncols])

        # ---- s-node groups (layer 1 only)
        if cfg["out_s"]:
            wb_bs = 0
            for g in range(len(types["bs"]["groups"])):
                m_bs, ncols = aggregate("bs", g, wb_bs)
                wb_bs += len(types["bs"]["groups"][g])
                t_x = spool.tile([P, GROUP], f32, tag="xg")
                nc.sync.dma_start(t_x[:, :ncols],
                                  d_xsT[:, g * GROUP:g * GROUP + ncols])
                ps2 = s2pool.tile([P, GROUP], f32, space="PSUM", tag="s2")
                nc.tensor.matmul(ps2[:, :ncols], lhsT=w_Wlbs,
                                 rhs=m_bs[:, :ncols], start=True, stop=False)
                nc.tensor.matmul(ps2[:, :ncols], lhsT=w_Wrs,
                                 rhs=t_x[:, :ncols], start=False, stop=True)
                t_o = spool.tile([P, GROUP], f32, tag="ob")
                nc.scalar.activation(out=t_o[:, :ncols], in_=ps2[:, :ncols],
                                     func=mybir.ActivationFunctionType.Lrelu,
                                     bias=w_bs, alpha=0.01)
                nc.sync.dma_start(d_nsT[:, g * GROUP:g * GROUP + ncols],
                                  t_o[:, :ncols])

    nc.compile()
    return nc


def _pack_weights(cfg, Wlbb, Wlsb, Wrb, bb, bs_bias=None, Wlbs=None, Wrs=None,
                  WhT=None, bh0=None):
    nw = 3 * D + (2 * D if cfg["out_s"] else 0) + (8 if cfg["head"] else 0) + S + 3
    w = np.zeros((P, nw), np.float32)
    off = 0
    for M in [Wlbb, Wlsb, Wrb]:
        w[:, off:off + D] = M; off += D
    if cfg["out_s"]:
        w[:, off:off + D] = Wlbs; off += D
        w[:, off:off + D] = Wrs; off += D
    if cfg["head"]:
        w[:, off:off + 8] = WhT; off += 8
    w[:, off:off + S] = np.arange(S, dtype=np.float32)[None, :]; off += S
    w[:, off] = bb; off += 1
    if bs_bias is not None:
        w[:, off] = bs_bias
    off += 1
    if bh0 is not None:
        w[:8, off] = bh0
    return w


LAST_HW_NS = None
LAST_EXEC_S = None


def kernel(x_b, x_s, Wl, bl, Wr, Wh, bh, ei_bb, ei_sb, ei_bs):
    x_b = np.asarray(x_b, np.float32); x_s = np.asarray(x_s, np.float32)
    Wl = np.asarray(Wl, np.float32); bl = np.asarray(bl, np.float32)
    Wr = np.asarray(Wr, np.float32); Wh = np.asarray(Wh, np.float32)
    bh = np.asarray(bh, np.float32)
    ei_bb = np.asarray(ei_bb); ei_sb = np.asarray(ei_sb); ei_bs = np.asarray(ei_bs)

    # ---------------- layer 1 prep (original node ids as gather indices)
    pc_bb = _shard_edges(ei_bb[0], ei_bb[1], NB)
    pc_sb = _shard_edges(ei_sb[0], ei_sb[1], NB)   # dst are b-nodes < NS
    pc_bs = _shard_edges(ei_bs[0], ei_bs[1], NS)
    i_bb, r_bb, v_bb, g_bb = _pack_type(pc_bb, NLB)
    i_sb, r_sb, v_sb, g_sb = _pack_type(pc_sb, NS // NCORES)
    i_bs, r_bs, v_bs, g_bs = _pack_type(pc_bs, NLS)

    cfgA = {
        "tabs": {"tab_b": NB, "tab_s": NS},
        "types": [
            {"name": "bb", "tab": "tab_b", "Wtot": i_bb.shape[2], "groups": g_bb},
            {"name": "sb", "tab": "tab_s", "Wtot": i_sb.shape[2], "groups": g_sb},
            {"name": "bs", "tab": "tab_b", "Wtot": i_bs.shape[2], "groups": g_bs},
        ],
        "head": False, "out_s": True,
    }
    ncA = _build_launch(cfgA)
    wA = _pack_weights(cfgA, Wl[0, 0], Wl[0, 1], Wr[0, 0] + Wr[0, 1],
                       bl[0, 0] + bl[0, 1], bs_bias=bl[0, 2],
                       Wlbs=Wl[0, 2], Wrs=Wr[0, 2])
    in_maps = []
    for c in range(NCORES):
        in_maps.append({
            "tab_b": x_b, "tab_s": x_s,
            "xbT": np.ascontiguousarray(x_b[c::NCORES].T),
            "xsT": np.ascontiguousarray(x_s[c::NCORES].T),
            "wts": wA,
            "idx_bb": i_bb[c], "rel_bb": r_bb[c], "ivc_bb": v_bb[c],
            "idx_sb": i_sb[c], "rel_sb": r_sb[c], "ivc_sb": v_sb[c],
            "idx_bs": i_bs[c], "rel_bs": r_bs[c], "ivc_bs": v_bs[c],
        })
    _tr = False
    _t0 = time.time()
    resA = run_bass_kernel_spmd(ncA, in_maps, core_ids=list(range(NCORES)),
                                trace=_tr, trace_cores=[0] if _tr else None)
    _execA = time.time() - _t0
    if _tr:
        print("launchA exec_ns:", resA.exec_time_ns,
              "trace:", (resA.instructions_and_trace or (None, None))[1], flush=True)
    nbT = [resA.results[c]["nbT"] for c in range(NCORES)]
    nsT = [resA.results[c]["nsT"] for c in range(NCORES)]

    # ---------------- layer 2: host halo exchange + index translation
    xb1 = np.concatenate([t.T for t in nbT], 0)   # [NB, D] core-block order
    xs1 = np.concatenate([t.T for t in nsT], 0)   # [NS, D]

    def tr_b(v):
        return (v % NCORES) * NLB + v // NCORES

    def tr_s(v):
        return (v % NCORES) * NLS + v // NCORES

    pc_bb2 = _shard_edges(tr_b(ei_bb[0]), ei_bb[1], NB)
    pc_sb2 = _shard_edges(tr_s(ei_sb[0]), ei_sb[1], NB)
    i_bb2, r_bb2, v_bb2, g_bb2 = _pack_type(pc_bb2, NLB)
    i_sb2, r_sb2, v_sb2, g_sb2 = _pack_type(pc_sb2, NS // NCORES)

    cfgB = {
        "tabs": {"tab_b": NB, "tab_s": NS},
        "types": [
            {"name": "bb", "tab": "tab_b", "Wtot": i_bb2.shape[2], "groups": g_bb2},
            {"name": "sb", "tab": "tab_s", "Wtot": i_sb2.shape[2], "groups": g_sb2},
        ],
        "head": True, "out_s": False,
    }
    ncB = _build_launch(cfgB)
    wB = _pack_weights(cfgB, Wl[1, 0], Wl[1, 1], Wr[1, 0] + Wr[1, 1],
                       bl[1, 0] + bl[1, 1], WhT=Wh.T, bh0=bh)
    in_mapsB = []
    for c in range(NCORES):
        in_mapsB.append({
            "tab_b": xb1, "tab_s": xs1,
            "xbT": nbT[c], "wts": wB,
            "idx_bb": i_bb2[c], "rel_bb": r_bb2[c], "ivc_bb": v_bb2[c],
            "idx_sb": i_sb2[c], "rel_sb": r_sb2[c], "ivc_sb": v_sb2[c],
        })
    _t0 = time.time()
    resB = run_bass_kernel_spmd(ncB, in_mapsB, core_ids=list(range(NCORES)),
                                trace=_tr, trace_cores=[0] if _tr else None)
    _execB = time.time() - _t0
    if _tr:
        print("launchB exec_ns:", resB.exec_time_ns,
              "trace:", (resB.instructions_and_trace or (None, None))[1], flush=True)
    global LAST_HW_NS, LAST_EXEC_S
    if resA.exec_time_ns and resB.exec_time_ns:
        LAST_HW_NS = int(resA.exec_time_ns) + int(resB.exec_time_ns)
    LAST_EXEC_S = (_execA, _execB)

    y = np.empty((NB, 8), np.float32)
    for c in range(NCORES):
        y[np.arange(NLB) * NCORES + c] = resB.results[c]["yT"].T
    return y
